# revision 8
# baseline (speedup 1.0000x reference)
"""Top-8-per-row kernel for x[2048, 32768] fp32 on 8 TRN2 NeuronCores.

Data-parallel over rows: 256 rows/core = 2 partition blocks of 128.
Raw-semaphore Bass program (no TileContext): the Sync engine's HWDGE
ring streams column tiles into SBUF at the per-core DMA line rate
(~26.3 B/ns per SDMA engine, 16 engines ~ 421 GB/s; measured to be a
hard cap). DVE MAX8 reduces each tile to its top-8 per partition, a
final MAX8 folds the per-tile candidates, writing through a reversed
view so the result is ascending in memory, and the Scalar ring stores
the result.

Measured decomposition of exec_time on a healthy device window:
~2.4 us preamble (const-AP memsets + barrier + first-DMA latency),
~80 us stream (per-engine DMA line rate; hard cap), ~2.0 us tail,
~8.6 us postamble (store issue + block barrier + the walrus sem-file
sweep, which is required for repeat-execution semantics and not
controllable from kernel code). The shared device also drifts between
healthy (~26 B/ns/engine) and degraded (~17-24 B/ns) windows lasting
minutes, which moves whole-run timings by 10-20%.

Tail design: the DVE reduces at ~1.04 ns/col (+~0.15 us/instr) while
the stream delivers at ~1.23 ns/col, and a tile's MAX8 can only start
~0.5 us after its last byte (DMA completion-semaphore latency). A tile
of size s with C columns still to stream after it therefore exposes
~1.04*s - 0.19*C + overheads of serial DVE work past the end of the
stream. Block 1's tile sizes taper so this term stays ~2 us for every
tile (numerically optimized), and the final 256 columns are DMA'd
straight into the tail of the candidate buffer so the single final
MAX8 over [17*8 cand cols | 256 raw cols] is both the last reduction
and the fold. The stores carry no completion wait: the postamble far
outlasts the ~1.5 us HBM write receipt, and st has no reader.
"""

from contextlib import ExitStack

import numpy as np

import concourse.bass as bass
from concourse import bacc, mybir
from concourse.bass_utils import run_bass_kernel_spmd

B = 2048
N = 32768
K = 8
N_CORES = 8
ROWS_PER_CORE = B // N_CORES  # 256
P = 128
N_BLOCKS = ROWS_PER_CORE // P  # 2
F32 = mybir.dt.float32

TILES0 = [4096] * 8  # block 0: full 16KB-line rate; tail fully hidden
# Block 1 tapers geometrically over its second half: the DVE reduces at
# ~1.04 ns/col while the stream delivers at ~1.23 ns/col, so the DVE
# only earns ~0.19 us of catch-up slack per 1000 streamed columns. A
# tile of size s with C columns still to stream after it serializes
# ~1.04*s - 0.19*C into the exposed tail, so tile sizes must shrink
# roughly in proportion to the columns remaining.
TILES1 = [
    2048, 3840, 3328, 2944, 2560, 2304, 2048, 1920, 1664,
    1536, 1408, 1280, 1280, 1152, 1152, 1024, 1024,
]
INLINE1 = 256  # final cols of block 1, DMA'd into cands1's tail
assert sum(TILES0) == N and sum(TILES1) + INLINE1 == N
BUFS = 8


def _build(bufs: int = BUFS) -> bass.Bass:
    tapers = [list(TILES0), list(TILES1)]
    max_c = max(max(tp) for tp in tapers)

    nc = bacc.Bacc(
        "TRN2", target_bir_lowering=False, debug=False, num_devices=N_CORES
    )
    x = nc.dram_tensor("x", [ROWS_PER_CORE, N], F32, kind="ExternalInput").ap()
    out = nc.dram_tensor("out", [ROWS_PER_CORE, K], F32, kind="ExternalOutput").ap()

    tiles = [(b, t) for b in range(N_BLOCKS) for t in range(len(tapers[b]))]
    n_tiles = len(tiles)
    n_tiles1 = len(tapers[1])
    # Per-slot DMA-completion thresholds: each dma_start bumps its slot
    # sem by 16 (one per SDMA engine); a single shared counting sem
    # would be racy across concurrently-draining transfers.
    slot_used = [0] * bufs
    thresh = []
    for i in range(n_tiles):
        s = i % bufs
        slot_used[s] += 1
        thresh.append(16 * slot_used[s])

    cand_cols = [len(tapers[0]) * K, n_tiles1 * K + INLINE1]

    with ExitStack() as ctx:
        block = ctx.enter_context(nc.Block())
        ld = [ctx.enter_context(nc.semaphore(f"ld{s}")) for s in range(bufs)]
        li = ctx.enter_context(nc.semaphore("li"))
        vd = ctx.enter_context(nc.semaphore("vd"))
        res = ctx.enter_context(nc.semaphore("res"))
        st = ctx.enter_context(nc.semaphore("st"))
        data = [
            ctx.enter_context(nc.sbuf_tensor(f"data{s}", [P, max_c], F32))
            for s in range(bufs)
        ]
        cands = [
            ctx.enter_context(nc.sbuf_tensor(f"cands{b}", [P, cand_cols[b]], F32))
            for b in range(N_BLOCKS)
        ]
        asc = [
            ctx.enter_context(nc.sbuf_tensor(f"asc{b}", [P, K], F32))
            for b in range(N_BLOCKS)
        ]

        @block.sync
        def _(eng: bass.BassEngine):
            for i, (b, t) in enumerate(tiles):
                s = i % bufs
                rows = slice(b * P, (b + 1) * P)
                tp = tapers[b]
                off, sz = sum(tp[:t]), tp[t]
                if i >= bufs:
                    eng.wait_ge(vd, i - bufs + 1)
                eng.dma_start(
                    out=data[s][:, :sz], in_=x[rows, off : off + sz]
                ).then_inc(ld[s], 16)
            # Last-landing transfer: block 1's final columns straight into
            # the tail of cands1, so one MAX8 reduces + folds them.
            rows = slice(P, 2 * P)
            eng.dma_start(
                out=cands[1][:, n_tiles1 * K :],
                in_=x[rows, N - INLINE1 :],
            ).then_inc(li, 16)

        @block.vector
        def _(vec: bass.BassVectorEngine):
            done = 0
            for i, (b, t) in enumerate(tiles):
                s = i % bufs
                tp = tapers[b]
                sz = tp[t]
                vec.wait_ge(ld[s], thresh[i])
                vec.max(cands[b][:, t * K : (t + 1) * K], data[s][:, :sz]).then_inc(
                    vd, 1
                )
                done += 1
                if b == 0 and t == len(tp) - 1:
                    # DVE writes drain asynchronously: same-engine RAW
                    # needs the sem wait for visibility, not just
                    # program order.
                    vec.wait_ge(vd, done)
                    # Descending MAX8 into a reversed view = ascending
                    # in memory; skips a sem round-trip and a copy.
                    vec.max(asc[0][:, ::-1], cands[0][:]).then_inc(res, 1)
            # Block 1 fold: candidates of the reduced tiles plus the
            # raw inline columns, one MAX8. vd wait for same-engine write
            # visibility; li wait for the inline DMA.
            vec.wait_ge(vd, done)
            vec.wait_ge(li, 16)
            vec.max(asc[1][:, ::-1], cands[1][:]).then_inc(res, 1)

        @block.scalar
        def _(eng: bass.BassEngine):
            # No completion wait on the stores: the walrus postamble
            # (~7.5 us of sem resets) runs after the block barrier and far
            # outlasts the ~1.5 us HBM write receipt, so the data is long
            # landed before the program halts. st has no reader, so a
            # late inc racing the postamble sem sweep is harmless.
            for b in range(N_BLOCKS):
                rows = slice(b * P, (b + 1) * P)
                eng.wait_ge(res, b + 1)
                eng.dma_start(
                    out=out[rows, :], in_=asc[b][:], single_packet=True
                ).then_inc(st, 16)

    nc.compile()
    return nc


def kernel(x: np.ndarray, k) -> np.ndarray:
    k = int(np.asarray(k))
    assert k == K, f"kernel hardcoded for k={K}, got {k}"
    assert x.shape == (B, N), x.shape
    x = np.ascontiguousarray(x, dtype=np.float32)

    nc = _build()
    in_maps = [
        {"x": x[c * ROWS_PER_CORE : (c + 1) * ROWS_PER_CORE]} for c in range(N_CORES)
    ]
    res = run_bass_kernel_spmd(nc, in_maps, list(range(N_CORES)))
    out = np.concatenate([res.results[c]["out"] for c in range(N_CORES)], axis=0)
    return np.asarray(out, dtype=np.float32)


if __name__ == "__main__":
    rng = np.random.default_rng(0)
    xs = rng.standard_normal((B, N), dtype=np.float32)
    got = kernel(xs, 8)
    want = np.sort(xs, axis=1)[:, -K:]
    err = np.max(np.abs(got - want))
    print("absmax err:", err)


# revision 9
# speedup vs baseline: 1.1079x; 1.1079x over previous
"""Top-8-per-row kernel for x[2048, 32768] fp32 on 8 TRN2 NeuronCores.

Data-parallel over rows: 256 rows/core = 2 partition blocks of 128.
Raw-semaphore Bass program (no TileContext): the Sync engine's HWDGE
ring streams column tiles into SBUF at the per-core DMA line rate
(~26.3 B/ns per SDMA engine, 16 engines ~ 421 GB/s; measured to be a
hard cap). DVE MAX8 reduces each tile to its top-8 per partition, a
final MAX8 folds the per-tile candidates, writing through a reversed
view so the result is ascending in memory, and the Scalar ring stores
the result.

Measured decomposition of exec_time on a healthy device window:
~2.4 us preamble (const-AP memsets + barrier + first-DMA latency),
~80 us stream (per-engine DMA line rate; hard cap), ~2.0 us tail,
~8.6 us postamble (store issue + block barrier + the walrus sem-file
sweep, which is required for repeat-execution semantics and not
controllable from kernel code). The shared device also drifts between
healthy (~26 B/ns/engine) and degraded (~17-24 B/ns) windows lasting
minutes, which moves whole-run timings by 10-20%.

Tail design: the DVE reduces at ~1.04 ns/col (+~0.15 us/instr) while
the stream delivers at ~1.23 ns/col, and a tile's MAX8 can only start
~0.5 us after its last byte (DMA completion-semaphore latency). A tile
of size s with C columns still to stream after it therefore exposes
~1.04*s - 0.19*C + overheads of serial DVE work past the end of the
stream. Block 1's tile sizes taper so this term stays ~2 us for every
tile (numerically optimized), and the final 256 columns are DMA'd
straight into the tail of the candidate buffer so the single final
MAX8 over [17*8 cand cols | 256 raw cols] is both the last reduction
and the fold. The stores carry no completion wait: the postamble far
outlasts the ~1.5 us HBM write receipt, and st has no reader.
"""

from contextlib import ExitStack

import numpy as np

import concourse.bass as bass
from concourse import bacc, mybir
from concourse.bass_utils import run_bass_kernel_spmd

B = 2048
N = 32768
K = 8
N_CORES = 8
ROWS_PER_CORE = B // N_CORES  # 256
P = 128
N_BLOCKS = ROWS_PER_CORE // P  # 2
F32 = mybir.dt.float32

TILES0 = [4096] * 8  # block 0: full 16KB-line rate; tail fully hidden
# Block 1 tapers geometrically over its second half: the DVE reduces at
# ~1.04 ns/col while the stream delivers at ~1.23 ns/col, so the DVE
# only earns ~0.19 us of catch-up slack per 1000 streamed columns. A
# tile of size s with C columns still to stream after it serializes
# ~1.04*s - 0.19*C into the exposed tail, so tile sizes must shrink
# roughly in proportion to the columns remaining.
TILES1 = [
    2048, 3840, 3328, 2944, 2560, 2304, 2048, 1920, 1664,
    1536, 1408, 1280, 1280, 1152, 1152, 1024, 1024,
]
INLINE1 = 256  # final cols of block 1, DMA'd into cands1's tail
assert sum(TILES0) == N and sum(TILES1) + INLINE1 == N
BUFS = 8


def _build(bufs: int = BUFS) -> bass.Bass:
    tapers = [list(TILES0), list(TILES1)]
    max_c = max(max(tp) for tp in tapers)

    nc = bacc.Bacc(
        "TRN2", target_bir_lowering=False, debug=False, num_devices=N_CORES
    )
    x = nc.dram_tensor("x", [ROWS_PER_CORE, N], F32, kind="ExternalInput").ap()
    out = nc.dram_tensor("out", [ROWS_PER_CORE, K], F32, kind="ExternalOutput").ap()

    tiles = [(b, t) for b in range(N_BLOCKS) for t in range(len(tapers[b]))]
    n_tiles = len(tiles)
    n_tiles1 = len(tapers[1])
    # Per-slot DMA-completion thresholds: each dma_start bumps its slot
    # sem by 16 (one per SDMA engine); a single shared counting sem
    # would be racy across concurrently-draining transfers.
    slot_used = [0] * bufs
    thresh = []
    for i in range(n_tiles):
        s = i % bufs
        slot_used[s] += 1
        thresh.append(16 * slot_used[s])

    cand_cols = [len(tapers[0]) * K, n_tiles1 * K + INLINE1]

    with ExitStack() as ctx:
        block = ctx.enter_context(nc.Block())
        ld = [ctx.enter_context(nc.semaphore(f"ld{s}")) for s in range(bufs)]
        li = ctx.enter_context(nc.semaphore("li"))
        vd = ctx.enter_context(nc.semaphore("vd"))
        res = ctx.enter_context(nc.semaphore("res"))
        st = ctx.enter_context(nc.semaphore("st"))
        data = [
            ctx.enter_context(nc.sbuf_tensor(f"data{s}", [P, max_c], F32))
            for s in range(bufs)
        ]
        cands = [
            ctx.enter_context(nc.sbuf_tensor(f"cands{b}", [P, cand_cols[b]], F32))
            for b in range(N_BLOCKS)
        ]
        asc = [
            ctx.enter_context(nc.sbuf_tensor(f"asc{b}", [P, K], F32))
            for b in range(N_BLOCKS)
        ]

        @block.sync
        def _(eng: bass.BassEngine):
            for i, (b, t) in enumerate(tiles):
                s = i % bufs
                rows = slice(b * P, (b + 1) * P)
                tp = tapers[b]
                off, sz = sum(tp[:t]), tp[t]
                if i >= bufs:
                    eng.wait_ge(vd, i - bufs + 1)
                eng.dma_start(
                    out=data[s][:, :sz], in_=x[rows, off : off + sz]
                ).then_inc(ld[s], 16)
            # Last-landing transfer: block 1's final columns straight into
            # the tail of cands1, so one MAX8 reduces + folds them.
            rows = slice(P, 2 * P)
            eng.dma_start(
                out=cands[1][:, n_tiles1 * K :],
                in_=x[rows, N - INLINE1 :],
            ).then_inc(li, 16)

        @block.vector
        def _(vec: bass.BassVectorEngine):
            done = 0
            for i, (b, t) in enumerate(tiles):
                s = i % bufs
                tp = tapers[b]
                sz = tp[t]
                vec.wait_ge(ld[s], thresh[i])
                vec.max(cands[b][:, t * K : (t + 1) * K], data[s][:, :sz]).then_inc(
                    vd, 1
                )
                done += 1
                if b == 0 and t == len(tp) - 1:
                    # DVE writes drain asynchronously: same-engine RAW
                    # needs the sem wait for visibility, not just
                    # program order.
                    vec.wait_ge(vd, done)
                    # Descending MAX8 into a reversed view = ascending
                    # in memory; skips a sem round-trip and a copy.
                    vec.max(asc[0][:, ::-1], cands[0][:]).then_inc(res, 1)
            # Block 1 fold: candidates of the reduced tiles plus the
            # raw inline columns, one MAX8. vd wait for same-engine write
            # visibility; li wait for the inline DMA.
            vec.wait_ge(vd, done)
            vec.wait_ge(li, 16)
            vec.max(asc[1][:, ::-1], cands[1][:]).then_inc(res, 1)

        @block.scalar
        def _(eng: bass.BassEngine):
            # No completion wait on the stores: the walrus postamble
            # (~7.5 us of sem resets) runs after the block barrier and far
            # outlasts the ~1.5 us HBM write receipt, so the data is long
            # landed before the program halts. st has no reader, so a
            # late inc racing the postamble sem sweep is harmless.
            for b in range(N_BLOCKS):
                rows = slice(b * P, (b + 1) * P)
                eng.wait_ge(res, b + 1)
                eng.dma_start(out=out[rows, :], in_=asc[b][:]).then_inc(st, 16)

    nc.compile()
    return nc


def kernel(x: np.ndarray, k) -> np.ndarray:
    k = int(np.asarray(k))
    assert k == K, f"kernel hardcoded for k={K}, got {k}"
    assert x.shape == (B, N), x.shape
    x = np.ascontiguousarray(x, dtype=np.float32)

    nc = _build()
    in_maps = [
        {"x": x[c * ROWS_PER_CORE : (c + 1) * ROWS_PER_CORE]} for c in range(N_CORES)
    ]
    res = run_bass_kernel_spmd(nc, in_maps, list(range(N_CORES)))
    out = np.concatenate([res.results[c]["out"] for c in range(N_CORES)], axis=0)
    return np.asarray(out, dtype=np.float32)


if __name__ == "__main__":
    rng = np.random.default_rng(0)
    xs = rng.standard_normal((B, N), dtype=np.float32)
    got = kernel(xs, 8)
    want = np.sort(xs, axis=1)[:, -K:]
    err = np.max(np.abs(got - want))
    print("absmax err:", err)
